# revision 6
# baseline (speedup 1.0000x reference)
"""Trainium2 Bass kernel for MultiLatentAttention (MLA).

Sharding: 8 cores = 2 (batch) x 4 (head-groups of 4 heads).
Each core computes its batch's latents (replicated within the batch group),
its 4 heads' q/k/v up-projections + SDPA, and a partial output projection
y_part = attn_out @ Wo[:, heads].T.  Host sums the 4 partials per batch.

On-device layout is feature-major ("transposed"): activations are [feat, S]
so every matmul contracts along the partition dim with zero transposes.
Scores are computed transposed [k, q]; softmax denominator comes from a
ones-vector matmul; normalization uses a K=1 broadcast matmul.
All matmul operands are bf16 (f32 PSUM accumulation).
"""

import sys

if "/opt/trn_rl_repo" not in sys.path:
    sys.path.insert(0, "/opt/trn_rl_repo")

import numpy as np
import ml_dtypes

BF16 = ml_dtypes.bfloat16

B, S, D, H = 2, 2048, 2048, 16
QR, KVR = 1536, 512
NOPE, RD, VD = 128, 64, 128
QK_D = NOPE + RD
HL = 4          # heads per core
G = 4           # head groups
PAN = 512       # panel width
P = 128

_cache = {}


def _build_module():
    import concourse.bacc as bacc
    import concourse.mybir as mybir
    import concourse.tile as tile

    dt = mybir.dt
    f32, bf16 = dt.float32, dt.bfloat16
    AF = mybir.ActivationFunctionType

    nc = bacc.Bacc("TRN2", target_bir_lowering=False, debug=False, num_devices=8)

    def inp(name, shape, dtype=bf16):
        return nc.dram_tensor(name, shape, dtype, kind="ExternalInput").ap()

    xT = inp("xT", [D, S])                  # x[b].T
    wqd = inp("wqd", [D, QR])               # Wq_down.T
    wkvd = inp("wkvd", [D, KVR])            # Wkv_down.T
    wkr = inp("wkr", [D, RD])               # Wk_rope.T
    wqall = inp("wqall", [QR, 768])         # [Wq_up_g.T*s | Wq_rope_g.T*s]
    wku = inp("wku", [KVR, 512])            # Wk_up_g.T
    wvu = inp("wvu", [KVR, 512])            # Wv_up_g.T
    wo = inp("wo", [512, D])                # Wo[:, cols_g].T
    cosT = inp("cosT", [32, S], f32)
    sinT = inp("sinT", [32, S], f32)
    masks = inp("masks", [P, 4 * PAN])      # multiplicative causal masks
    onc = inp("onc", [P, 1])                # ones column
    onr = inp("onr", [1, P])                # ones row
    y = nc.dram_tensor("y", [S, D], f32, kind="ExternalOutput").ap()

    KT_D = D // P      # 16 k-tiles over model dim
    KT_QR = QR // P    # 12
    KT_KV = KVR // P   # 4
    NP = S // PAN      # 4 panels

    with tile.TileContext(nc) as tc:
        # ---- persistent pools -------------------------------------------
        with (
            tc.tile_pool(name="res", bufs=1) as res,
            tc.tile_pool(name="work", bufs=3) as work,
            tc.tile_pool(name="dram", bufs=1, space="DRAM") as dram,
        ):
            k_r_sb = res.tile([64, S], bf16, tag="k_r")
            k_c_sb = res.tile([P, HL, S], bf16, tag="k_c")
            v_sb = res.tile([P, S // P, 512], bf16, tag="v")
            ao_sb = res.tile([P, HL, S], bf16, tag="ao")
            masks_sb = res.tile([P, G, PAN], bf16, tag="masks")
            onc_sb = res.tile([P, 1], bf16, tag="onc")
            onr_sb = res.tile([1, P], bf16, tag="onr")
            nc.sync.dma_start(masks_sb[:], masks.rearrange("p (j q) -> p j q", q=PAN))
            nc.sync.dma_start(onc_sb[:], onc[:])
            nc.sync.dma_start(onr_sb[:], onr[:])

            cq_dram = [dram.tile([QR, PAN], bf16, tag=f"cq{n}", name=f"cq{n}") for n in range(NP)]
            qn_dram = [dram.tile([P, 4, PAN], bf16, tag=f"qn{n}", name=f"qn{n}") for n in range(NP)]
            qr_dram = [dram.tile([64, 4, PAN], bf16, tag=f"qr{n}", name=f"qr{n}") for n in range(NP)]

            # ---- phase A+B scope: latents + up-projections --------------
            with tc.tile_pool(name="ab", bufs=1) as ab:
                c_kv_sb = ab.tile([P, KT_KV, S], bf16, tag="c_kv")
                cos_sb = ab.tile([32, S], f32, tag="cos")
                sin_sb = ab.tile([32, S], f32, tag="sin")
                nc.sync.dma_start(cos_sb[:], cosT[:])
                nc.sync.dma_start(sin_sb[:], sinT[:])

                def rope_block(dst64, src64, cs, sn):
                    # dst/src are [64, PAN]; rows 0:32 = first half dims
                    t1 = work.tile([32, PAN], f32, tag="rope_t1")
                    t2 = work.tile([32, PAN], f32, tag="rope_t2")
                    nc.vector.tensor_mul(t1, src64[0:32, :], cs)
                    nc.vector.tensor_mul(t2, src64[32:64, :], sn)
                    nc.vector.tensor_sub(dst64[0:32, :], t1, t2)
                    t3 = work.tile([32, PAN], f32, tag="rope_t1")
                    t4 = work.tile([32, PAN], f32, tag="rope_t2")
                    nc.vector.tensor_mul(t3, src64[32:64, :], cs)
                    nc.vector.tensor_mul(t4, src64[0:32, :], sn)
                    nc.vector.tensor_add(dst64[32:64, :], t3, t4)

                # ---------------- Phase A: down projections --------------
                with (
                    tc.tile_pool(name="pa", bufs=2) as pa,
                    tc.tile_pool(name="paw", bufs=3) as paw,
                    tc.tile_pool(name="psA", bufs=4, space="PSUM") as psA,
                ):
                    wkvd_sb = pa.tile([P, KT_D, KVR], bf16, tag="wkvd")
                    nc.sync.dma_start(
                        wkvd_sb[:], wkvd.rearrange("(kt p) m -> p kt m", p=P)
                    )
                    wkr_sb = pa.tile([P, KT_D, RD], bf16, tag="wkr")
                    nc.sync.dma_start(
                        wkr_sb[:], wkr.rearrange("(kt p) m -> p kt m", p=P)
                    )
                    for n in range(NP):
                        ns = slice(n * PAN, (n + 1) * PAN)
                        x_sb = pa.tile([P, KT_D, PAN], bf16, tag="x")
                        nc.sync.dma_start(
                            x_sb[:], xT[:, ns].rearrange("(kt p) s -> p kt s", p=P)
                        )
                        # c_q m-tiles -> DRAM panel
                        for m in range(KT_QR):
                            wq_sb = paw.tile([P, KT_D, P], bf16, tag="wqd_m")
                            nc.sync.dma_start(
                                wq_sb[:],
                                wqd[:, m * P : (m + 1) * P].rearrange(
                                    "(kt p) m -> p kt m", p=P
                                ),
                            )
                            ps = psA.tile([P, PAN], f32, tag="psA")
                            for kt in range(KT_D):
                                nc.tensor.matmul(
                                    ps,
                                    lhsT=wq_sb[:, kt, :],
                                    rhs=x_sb[:, kt, :],
                                    start=(kt == 0),
                                    stop=(kt == KT_D - 1),
                                )
                            st = work.tile([P, PAN], bf16, tag="cq_st")
                            nc.vector.tensor_copy(st, ps)
                            nc.sync.dma_start(cq_dram[n][m * P : (m + 1) * P, :], st)
                        # c_kv -> SBUF resident
                        for m in range(KT_KV):
                            ps = psA.tile([P, PAN], f32, tag="psA")
                            for kt in range(KT_D):
                                nc.tensor.matmul(
                                    ps,
                                    lhsT=wkvd_sb[:, kt, m * P : (m + 1) * P],
                                    rhs=x_sb[:, kt, :],
                                    start=(kt == 0),
                                    stop=(kt == KT_D - 1),
                                )
                            nc.vector.tensor_copy(c_kv_sb[:, m, ns], ps)
                        # k_rope -> rope -> SBUF resident
                        ps = psA.tile([64, PAN], f32, tag="psKR")
                        for kt in range(KT_D):
                            nc.tensor.matmul(
                                ps,
                                lhsT=wkr_sb[:, kt, :],
                                rhs=x_sb[:, kt, :],
                                start=(kt == 0),
                                stop=(kt == KT_D - 1),
                            )
                        rope_block(k_r_sb[:, ns], ps, cos_sb[:, ns], sin_sb[:, ns])

                # ---------------- Phase B: up projections ----------------
                with (
                    tc.tile_pool(name="pb", bufs=1) as pb,
                    tc.tile_pool(name="pbc", bufs=2) as pbc,
                    tc.tile_pool(name="psB", bufs=4, space="PSUM") as psB,
                ):
                    wqall_sb = pb.tile([P, KT_QR, 768], bf16, tag="wqall")
                    nc.sync.dma_start(
                        wqall_sb[:], wqall.rearrange("(kt p) m -> p kt m", p=P)
                    )
                    wku_sb = pb.tile([P, KT_KV, 512], bf16, tag="wku")
                    nc.sync.dma_start(
                        wku_sb[:], wku.rearrange("(kt p) m -> p kt m", p=P)
                    )
                    wvu_sb = pb.tile([P, KT_KV, 512], bf16, tag="wvu")
                    nc.sync.dma_start(
                        wvu_sb[:], wvu.rearrange("(kt p) m -> p kt m", p=P)
                    )
                    # q (nope + rope), panel by panel, staged to DRAM
                    for n in range(NP):
                        ns = slice(n * PAN, (n + 1) * PAN)
                        cq_sb = pbc.tile([P, KT_QR, PAN], bf16, tag="cq_rd")
                        nc.sync.dma_start(
                            cq_sb[:],
                            cq_dram[n].rearrange("(kt p) s -> p kt s", p=P),
                        )
                        qst = pbc.tile([P, 4, PAN], bf16, tag="q_st")
                        for m in range(4):
                            ps = psB.tile([P, PAN], f32, tag="psB")
                            for kt in range(KT_QR):
                                nc.tensor.matmul(
                                    ps,
                                    lhsT=wqall_sb[:, kt, m * P : (m + 1) * P],
                                    rhs=cq_sb[:, kt, :],
                                    start=(kt == 0),
                                    stop=(kt == KT_QR - 1),
                                )
                            nc.vector.tensor_copy(qst[:, m, :], ps)
                        nc.sync.dma_start(qn_dram[n][:], qst[:])
                        # rope heads: one M=64 matmul per head so each lands
                        # at base partition 0 (PE requires matching bases)
                        qrst = pbc.tile([64, 4, PAN], bf16, tag="qr_st")
                        for h in range(HL):
                            c0 = 512 + 64 * h
                            ps = psB.tile([64, PAN], f32, tag="psBr64", bufs=2)
                            for kt in range(KT_QR):
                                nc.tensor.matmul(
                                    ps,
                                    lhsT=wqall_sb[:, kt, c0 : c0 + 64],
                                    rhs=cq_sb[:, kt, :],
                                    start=(kt == 0),
                                    stop=(kt == KT_QR - 1),
                                )
                            rope_block(
                                qrst[:, h, :], ps, cos_sb[:, ns], sin_sb[:, ns]
                            )
                        nc.sync.dma_start(qr_dram[n][:], qrst[:])
                    # k_c resident
                    for n in range(NP):
                        ns = slice(n * PAN, (n + 1) * PAN)
                        for m in range(HL):
                            ps = psB.tile([P, PAN], f32, tag="psB")
                            for kt in range(KT_KV):
                                nc.tensor.matmul(
                                    ps,
                                    lhsT=wku_sb[:, kt, m * P : (m + 1) * P],
                                    rhs=c_kv_sb[:, kt, ns],
                                    start=(kt == 0),
                                    stop=(kt == KT_KV - 1),
                                )
                            nc.vector.tensor_copy(k_c_sb[:, m, ns], ps)
                    # v resident: [S_tile, 4*VD]
                    for st in range(S // P):
                        ps = psB.tile([P, PAN], f32, tag="psB")
                        for kt in range(KT_KV):
                            nc.tensor.matmul(
                                ps,
                                lhsT=c_kv_sb[:, kt, st * P : (st + 1) * P],
                                rhs=wvu_sb[:, kt, :],
                                start=(kt == 0),
                                stop=(kt == KT_KV - 1),
                            )
                        nc.vector.tensor_copy(v_sb[:, st, :], ps)

            # ---------------- Phase C: SDPA ------------------------------
            with (
                tc.tile_pool(name="pq", bufs=2) as pq,
                tc.tile_pool(name="pe", bufs=4) as pe,
                tc.tile_pool(name="psS", bufs=3, space="PSUM") as psS,
                tc.tile_pool(name="psO", bufs=2, space="PSUM") as psO,
                tc.tile_pool(name="psDn", bufs=2, space="PSUM") as psDn,
                tc.tile_pool(name="psBr", bufs=1, space="PSUM") as psBr,
            ):
                for g in range(G):
                    gs = slice(g * PAN, (g + 1) * PAN)
                    qg = pq.tile([P, 4, PAN], bf16, tag="qg")
                    nc.sync.dma_start(qg[:], qn_dram[g][:])
                    qgr = pq.tile([64, 4, PAN], bf16, tag="qgr")
                    nc.sync.dma_start(qgr[:], qr_dram[g][:])
                    for h in range(HL):
                        qn = qg[:, h, :]
                        qr = qgr[:, h, :]
                        ps_o = psO.tile([P, PAN], f32, tag="ps_o")
                        ps_d = psDn.tile([1, PAN], f32, tag="ps_d")
                        nk = 4 * (g + 1)
                        for kb in range(nk):
                            ks = slice(kb * P, (kb + 1) * P)
                            ps_s = psS.tile([P, PAN], f32, tag="ps_s")
                            nc.tensor.matmul(
                                ps_s, lhsT=k_c_sb[:, h, ks], rhs=qn,
                                start=True, stop=False,
                            )
                            nc.tensor.matmul(
                                ps_s, lhsT=k_r_sb[:, ks], rhs=qr,
                                start=False, stop=True,
                            )
                            e = pe.tile([P, PAN], bf16, tag="e")
                            nc.scalar.activation(e, ps_s, AF.Exp)
                            if kb >= 4 * g:
                                nc.vector.tensor_mul(
                                    e, e, masks_sb[:, kb - 4 * g, :]
                                )
                            nc.tensor.matmul(
                                ps_o, lhsT=v_sb[:, kb, h * P : (h + 1) * P], rhs=e,
                                start=(kb == 0), stop=(kb == nk - 1),
                            )
                            nc.tensor.matmul(
                                ps_d, lhsT=onc_sb[:], rhs=e,
                                start=(kb == 0), stop=(kb == nk - 1),
                            )
                        rc = work.tile([1, PAN], f32, tag="rc")
                        nc.vector.reciprocal(rc, ps_d)
                        rcb = work.tile([1, PAN], bf16, tag="rcb")
                        nc.vector.tensor_copy(rcb, rc)
                        ps_b = psBr.tile([P, PAN], f32, tag="ps_b")
                        nc.tensor.matmul(
                            ps_b, lhsT=onr_sb[:], rhs=rcb, start=True, stop=True
                        )
                        bb = work.tile([P, PAN], f32, tag="bb")
                        nc.scalar.copy(bb, ps_b)
                        nc.vector.tensor_mul(ao_sb[:, h, gs], ps_o, bb)

            # ---------------- Phase D: output projection -----------------
            with (
                tc.tile_pool(name="pd", bufs=1) as pd,
                tc.tile_pool(name="psD", bufs=4, space="PSUM") as psD,
            ):
                wo_sb = pd.tile([P, HL, D], bf16, tag="wo")
                nc.sync.dma_start(wo_sb[:], wo.rearrange("(kt p) m -> p kt m", p=P))
                for m in range(S // P):
                    ms = slice(m * P, (m + 1) * P)
                    for nn in range(D // PAN):
                        ps = psD.tile([P, PAN], f32, tag="psD")
                        for kt in range(HL):
                            nc.tensor.matmul(
                                ps,
                                lhsT=ao_sb[:, kt, ms],
                                rhs=wo_sb[:, kt, nn * PAN : (nn + 1) * PAN],
                                start=(kt == 0),
                                stop=(kt == HL - 1),
                            )
                        yst = work.tile([P, PAN], f32, tag="y_st")
                        nc.vector.tensor_copy(yst, ps)
                        nc.sync.dma_start(
                            y[ms, nn * PAN : (nn + 1) * PAN], yst
                        )

    nc.compile()
    return nc


def _prep_inputs(x, positions, Wq_down, Wq_up, Wq_rope, Wkv_down, Wk_up, Wv_up,
                 Wk_rope, Wo):
    scale = np.float32(QK_D ** -0.5)
    bf = lambda a: np.ascontiguousarray(a).astype(BF16)

    shared = {
        "wqd": bf(Wq_down.T),
        "wkvd": bf(Wkv_down.T),
        "wkr": bf(Wk_rope.T),
        "onc": np.ones((P, 1), BF16),
        "onr": np.ones((1, P), BF16),
    }
    inv_freq = 1.0 / (10000.0 ** (np.arange(0, RD, 2, dtype=np.float32) / RD))
    ang = positions.astype(np.float32)[:, None] * inv_freq  # (S, 32)
    shared["cosT"] = np.ascontiguousarray(np.cos(ang).T).astype(np.float32)
    shared["sinT"] = np.ascontiguousarray(np.sin(ang).T).astype(np.float32)

    mk = np.zeros((P, G * PAN), np.float32)
    for j in range(G):
        p = np.arange(P)[:, None]
        q = np.arange(PAN)[None, :]
        mk[:, j * PAN : (j + 1) * PAN] = (j * P + p <= q).astype(np.float32)
    shared["masks"] = mk.astype(BF16)

    per_g = []
    for g in range(G):
        rs, rr = slice(512 * g, 512 * (g + 1)), slice(256 * g, 256 * (g + 1))
        per_g.append({
            "wqall": bf(np.concatenate(
                [(Wq_up[rs] * scale).T, (Wq_rope[rr] * scale).T], axis=1)),
            "wku": bf(Wk_up[rs].T),
            "wvu": bf(Wv_up[rs].T),
            "wo": bf(Wo[:, rs].T),
        })
    xT = [bf(x[b].T) for b in range(B)]

    in_maps = []
    for c in range(8):
        b, g = c // G, c % G
        m = dict(shared)
        m.update(per_g[g])
        m["xT"] = xT[b]
        in_maps.append(m)
    return in_maps


def kernel(**inputs):
    from concourse.bass_utils import run_bass_kernel_spmd

    if "nc" not in _cache:
        _cache["nc"] = _build_module()
    nc = _cache["nc"]

    in_maps = _prep_inputs(**inputs)
    res = run_bass_kernel_spmd(nc, in_maps, core_ids=list(range(8)))
    out = np.zeros((B, S, D), np.float32)
    for c in range(8):
        out[c // G] += res.results[c]["y"]
    return out


# revision 11
# speedup vs baseline: 1.2038x; 1.2038x over previous
"""Trainium2 Bass kernel for MultiLatentAttention (MLA).

Sharding: 8 cores = 2 (batch) x 4 (head-groups of 4 heads).
Within each batch group of 4 cores, the down-projections are sharded by
output rows and AllGathered (per S-panel, pipelined); the shared k_rope
head is sharded by S-panel and gathered once early.  Each core then runs
its 4 heads' up-projections + SDPA and a partial output projection
y_part = attn_out @ Wo[:, heads].T.  Host sums the 4 partials per batch.

On-device layout is feature-major ("transposed"): activations are [feat, S]
so every matmul contracts along the partition dim with zero transposes.
Scores are computed transposed [k, q]; softmax denominator comes from a
ones-vector matmul; normalization uses a K=1 broadcast matmul.
All matmul operands are bf16 (f32 PSUM accumulation).
"""

import sys

if "/opt/trn_rl_repo" not in sys.path:
    sys.path.insert(0, "/opt/trn_rl_repo")

import numpy as np
import ml_dtypes

BF16 = ml_dtypes.bfloat16

B, S, D, H = 2, 2048, 2048, 16
QR, KVR = 1536, 512
NOPE, RD, VD = 128, 64, 128
QK_D = NOPE + RD
HL = 4          # heads per core
G = 4           # head groups (= cores per batch group)
QSH = QR // G   # 384 c_q rows per core
KSH = KVR // G  # 128 c_kv rows per core
PAN = 512       # panel width
P = 128

_cache = {}


def _build_module():
    import concourse.bacc as bacc
    import concourse.mybir as mybir
    import concourse.tile as tile

    dt = mybir.dt
    f32, bf16 = dt.float32, dt.bfloat16
    AF = mybir.ActivationFunctionType

    nc = bacc.Bacc("TRN2", target_bir_lowering=False, debug=False, num_devices=8)

    def inp(name, shape, dtype=bf16):
        return nc.dram_tensor(name, shape, dtype, kind="ExternalInput").ap()

    xT = inp("xT", [D, S])                  # x[b].T
    xkr = inp("xkr", [D, PAN])              # x[b].T[:, my panel]
    wqd = inp("wqd", [D, QSH])              # Wq_down.T column slice
    wkvd = inp("wkvd", [D, KSH])            # Wkv_down.T column slice
    wkr = inp("wkr", [D, RD])               # Wk_rope.T
    wqall = inp("wqall", [QR, 768])         # [Wq_up_g.T*s | Wq_rope_g.T*s]
    wku = inp("wku", [KVR, 512])            # Wk_up_g.T
    wvu = inp("wvu", [KVR, 512])            # Wv_up_g.T
    wo = inp("wo", [512, D])                # Wo[:, cols_g].T
    cosT = inp("cosT", [32, S], f32)
    sinT = inp("sinT", [32, S], f32)
    coskr = inp("coskr", [32, PAN], f32)    # cos/sin for my k_rope panel
    sinkr = inp("sinkr", [32, PAN], f32)
    masks = inp("masks", [P, 4 * PAN])      # multiplicative causal masks
    onc = inp("onc", [P, 1])                # ones column
    onr = inp("onr", [1, P])                # ones row
    y = nc.dram_tensor("y", [S, D], f32, kind="ExternalOutput").ap()

    KT_D = D // P      # 16 k-tiles over model dim
    KT_QR = QR // P    # 12
    KT_KV = KVR // P   # 4
    NP = S // PAN      # 4 panels
    GROUPS = [[0, 1, 2, 3], [4, 5, 6, 7]]

    with tile.TileContext(nc) as tc:
        with (
            tc.tile_pool(name="res", bufs=1) as res,
            tc.tile_pool(name="panels", bufs=2) as panels,
            tc.tile_pool(name="work", bufs=2) as work,
            tc.tile_pool(name="dram", bufs=1, space="DRAM") as dram,
        ):
            # ---- SBUF residents for SDPA --------------------------------
            qn_sb = res.tile([P, HL, S], bf16, tag="qn")
            qr_sb = res.tile([64, HL, S], bf16, tag="qr")
            k_c_sb = res.tile([P, HL, S], bf16, tag="k_c")
            v_sb = res.tile([P, S // P, 512], bf16, tag="v")
            k_r_sb = res.tile([64, NP, PAN], bf16, tag="k_r")
            masks_sb = res.tile([P, G, PAN], bf16, tag="masks")
            onc_sb = res.tile([P, 1], bf16, tag="onc")
            onr_sb = res.tile([1, P], bf16, tag="onr")
            nc.sync.dma_start(masks_sb[:], masks.rearrange("p (j q) -> p j q", q=PAN))
            nc.sync.dma_start(onc_sb[:], onc[:])
            nc.sync.dma_start(onr_sb[:], onr[:])

            # ---- DRAM staging -------------------------------------------
            ag_in = [dram.tile([QSH + KSH, PAN], bf16, tag=f"agi{n}", name=f"agi{n}")
                     for n in range(NP)]
            ag_out = [dram.tile([G * (QSH + KSH), PAN], bf16, tag=f"ago{n}",
                                name=f"ago{n}") for n in range(NP)]
            kr_in = dram.tile([64, PAN], bf16, tag="kri", name="kri")
            kr_out = dram.tile([G * 64, PAN], bf16, tag="kro", name="kro")
            ao_dram = dram.tile([HL * P, S], bf16, tag="aod", name="aod")

            def rope_block(dst64, src64, cs, sn):
                # dst/src are [64, PAN]; rows 0:32 = first half dims
                t1 = work.tile([32, PAN], f32, tag="rope_t1")
                t2 = work.tile([32, PAN], f32, tag="rope_t2")
                nc.vector.tensor_mul(t1, src64[0:32, :], cs)
                nc.vector.tensor_mul(t2, src64[32:64, :], sn)
                nc.vector.tensor_sub(dst64[0:32, :], t1, t2)
                t3 = work.tile([32, PAN], f32, tag="rope_t1")
                t4 = work.tile([32, PAN], f32, tag="rope_t2")
                nc.vector.tensor_mul(t3, src64[32:64, :], cs)
                nc.vector.tensor_mul(t4, src64[0:32, :], sn)
                nc.vector.tensor_add(dst64[32:64, :], t3, t4)

            # ---- Phase A + B, panel-interleaved -------------------------
            with (
                tc.tile_pool(name="pa", bufs=1) as pa,
                tc.tile_pool(name="pb", bufs=1) as pb,
                tc.tile_pool(name="pbc", bufs=2) as pbc,
                tc.tile_pool(name="psA", bufs=3, space="PSUM") as psA,
                tc.tile_pool(name="psB", bufs=3, space="PSUM") as psB,
            ):
                # -- k_rope for my panel first, so the small gather clears early
                xkr_sb = panels.tile([P, KT_D, PAN], bf16, tag="panel", name="xkr_sb")
                nc.sync.dma_start(xkr_sb[:], xkr.rearrange("(kt p) s -> p kt s", p=P))
                wkr_sb = pa.tile([P, KT_D, RD], bf16, tag="wkr")
                nc.sync.dma_start(wkr_sb[:], wkr.rearrange("(kt p) m -> p kt m", p=P))
                ckr_sb = pa.tile([32, PAN], f32, tag="ckr")
                skr_sb = pa.tile([32, PAN], f32, tag="skr")
                nc.sync.dma_start(ckr_sb[:], coskr[:])
                nc.sync.dma_start(skr_sb[:], sinkr[:])
                ps = psA.tile([64, PAN], f32, tag="psKR", bufs=1)
                for kt in range(KT_D):
                    nc.tensor.matmul(
                        ps, lhsT=wkr_sb[:, kt, :], rhs=xkr_sb[:, kt, :],
                        start=(kt == 0), stop=(kt == KT_D - 1),
                    )
                krst = work.tile([64, PAN], bf16, tag="krst", bufs=1)
                rope_block(krst, ps, ckr_sb, skr_sb)
                nc.sync.dma_start(kr_in[:], krst)
                nc.gpsimd.collective_compute(
                    "AllGather", mybir.AluOpType.bypass,
                    replica_groups=GROUPS,
                    ins=[kr_in.opt()], outs=[kr_out.opt()],
                )
                nc.sync.dma_start(
                    k_r_sb[:], kr_out.rearrange("(g d) s -> d g s", d=64)
                )

                # -- weights
                wqd_sb = pa.tile([P, KT_D, QSH], bf16, tag="wqd")
                nc.sync.dma_start(wqd_sb[:], wqd.rearrange("(kt p) m -> p kt m", p=P))
                wkvd_sb = pa.tile([P, KT_D, KSH], bf16, tag="wkvd")
                nc.sync.dma_start(wkvd_sb[:], wkvd.rearrange("(kt p) m -> p kt m", p=P))
                wqall_sb = pb.tile([P, KT_QR, 768], bf16, tag="wqall")
                nc.sync.dma_start(wqall_sb[:], wqall.rearrange("(kt p) m -> p kt m", p=P))
                wku_sb = pb.tile([P, KT_KV, 512], bf16, tag="wku")
                nc.sync.dma_start(wku_sb[:], wku.rearrange("(kt p) m -> p kt m", p=P))
                wvu_sb = pb.tile([P, KT_KV, 512], bf16, tag="wvu")
                nc.sync.dma_start(wvu_sb[:], wvu.rearrange("(kt p) m -> p kt m", p=P))

                def phase_a(n):
                    """my slices of c_q / c_kv for panel n, then gather"""
                    ns = slice(n * PAN, (n + 1) * PAN)
                    x_sb = panels.tile([P, KT_D, PAN], bf16, tag="panel",
                                       name=f"x_sb{n}")
                    nc.sync.dma_start(
                        x_sb[:], xT[:, ns].rearrange("(kt p) s -> p kt s", p=P)
                    )
                    for m in range(QSH // P):  # 3 c_q row-tiles
                        ps = psA.tile([P, PAN], f32, tag="psA")
                        for kt in range(KT_D):
                            nc.tensor.matmul(
                                ps,
                                lhsT=wqd_sb[:, kt, m * P : (m + 1) * P],
                                rhs=x_sb[:, kt, :],
                                start=(kt == 0), stop=(kt == KT_D - 1),
                            )
                        st = work.tile([P, PAN], bf16, tag="cq_st")
                        nc.vector.tensor_copy(st, ps)
                        nc.sync.dma_start(ag_in[n][m * P : (m + 1) * P, :], st)
                    ps = psA.tile([P, PAN], f32, tag="psA")  # 1 c_kv row-tile
                    for kt in range(KT_D):
                        nc.tensor.matmul(
                            ps, lhsT=wkvd_sb[:, kt, :], rhs=x_sb[:, kt, :],
                            start=(kt == 0), stop=(kt == KT_D - 1),
                        )
                    st = work.tile([P, PAN], bf16, tag="cq_st")
                    nc.vector.tensor_copy(st, ps)
                    nc.sync.dma_start(ag_in[n][QSH : QSH + KSH, :], st)
                    nc.gpsimd.collective_compute(
                        "AllGather", mybir.AluOpType.bypass,
                        replica_groups=GROUPS,
                        ins=[ag_in[n].opt()], outs=[ag_out[n].opt()],
                    )

                def phase_b(n):
                    """up-projections for panel n from the gathered latents"""
                    ns = slice(n * PAN, (n + 1) * PAN)
                    # gathered latents: [(gi r p), s] with r=0..2 c_q, r=3 c_kv
                    gat = ag_out[n].rearrange("(gi r p) s -> p gi r s", p=P, r=4)
                    cq_sb = panels.tile([P, G, 3, PAN], bf16, tag="panel",
                                        name=f"cq_sb{n}")
                    for gi in range(G):
                        nc.sync.dma_start(cq_sb[:, gi, :, :], gat[:, gi, 0:3, :])
                    ckv_sb = pbc.tile([P, KT_KV, PAN], bf16, tag="ckv")
                    nc.sync.dma_start(ckv_sb[:], gat[:, :, 3, :])
                    cosp = pbc.tile([32, PAN], f32, tag="cosp", bufs=1)
                    sinp = pbc.tile([32, PAN], f32, tag="sinp", bufs=1)
                    nc.sync.dma_start(cosp[:], cosT[:, ns])
                    nc.sync.dma_start(sinp[:], sinT[:, ns])
                    for m in range(4):  # q nope heads
                        ps = psB.tile([P, PAN], f32, tag="psB")
                        for kt in range(KT_QR):
                            nc.tensor.matmul(
                                ps,
                                lhsT=wqall_sb[:, kt, m * P : (m + 1) * P],
                                rhs=cq_sb[:, kt // 3, kt % 3, :],
                                start=(kt == 0), stop=(kt == KT_QR - 1),
                            )
                        nc.vector.tensor_copy(qn_sb[:, m, ns], ps)
                    # rope heads: one M=64 matmul per head so each lands
                    # at base partition 0 (PE requires matching bases)
                    for h in range(HL):
                        c0 = 512 + 64 * h
                        ps = psB.tile([64, PAN], f32, tag="psBr64", bufs=1)
                        for kt in range(KT_QR):
                            nc.tensor.matmul(
                                ps,
                                lhsT=wqall_sb[:, kt, c0 : c0 + 64],
                                rhs=cq_sb[:, kt // 3, kt % 3, :],
                                start=(kt == 0), stop=(kt == KT_QR - 1),
                            )
                        rope_block(qr_sb[:, h, ns], ps, cosp, sinp)
                    # k_c for this panel
                    for m in range(HL):
                        ps = psB.tile([P, PAN], f32, tag="psB")
                        for kt in range(KT_KV):
                            nc.tensor.matmul(
                                ps,
                                lhsT=wku_sb[:, kt, m * P : (m + 1) * P],
                                rhs=ckv_sb[:, kt, :],
                                start=(kt == 0), stop=(kt == KT_KV - 1),
                            )
                        nc.vector.tensor_copy(k_c_sb[:, m, ns], ps)
                    # v for this panel's S-tiles
                    for sti in range(4):
                        st = 4 * n + sti
                        ps = psB.tile([P, PAN], f32, tag="psB")
                        for kt in range(KT_KV):
                            nc.tensor.matmul(
                                ps,
                                lhsT=ckv_sb[:, kt, sti * P : (sti + 1) * P],
                                rhs=wvu_sb[:, kt, :],
                                start=(kt == 0), stop=(kt == KT_KV - 1),
                            )
                        nc.vector.tensor_copy(v_sb[:, st, :], ps)

                # interleave emission so the shared panel slots rotate A/B/A/B
                phase_a(0)
                phase_a(1)
                phase_b(0)
                phase_a(2)
                phase_b(1)
                phase_a(3)
                phase_b(2)
                phase_b(3)

            # ---------------- Phase C: SDPA ------------------------------
            with (
                tc.tile_pool(name="pe", bufs=4) as pe,
                tc.tile_pool(name="psS", bufs=3, space="PSUM") as psS,
                tc.tile_pool(name="psO", bufs=2, space="PSUM") as psO,
                tc.tile_pool(name="psDn", bufs=2, space="PSUM") as psDn,
                tc.tile_pool(name="psBr", bufs=1, space="PSUM") as psBr,
            ):
                for g in range(G):
                    gs = slice(g * PAN, (g + 1) * PAN)
                    for h in range(HL):
                        qn = qn_sb[:, h, gs]
                        qr = qr_sb[:, h, gs]
                        ps_o = psO.tile([P, PAN], f32, tag="ps_o")
                        ps_d = psDn.tile([1, PAN], f32, tag="ps_d")
                        nk = 4 * (g + 1)
                        for kb in range(nk):
                            ks = slice(kb * P, (kb + 1) * P)
                            ps_s = psS.tile([P, PAN], f32, tag="ps_s")
                            nc.tensor.matmul(
                                ps_s, lhsT=k_c_sb[:, h, ks], rhs=qn,
                                start=True, stop=False,
                            )
                            nc.tensor.matmul(
                                ps_s,
                                lhsT=k_r_sb[:, kb // 4, (kb % 4) * P : (kb % 4 + 1) * P],
                                rhs=qr,
                                start=False, stop=True,
                            )
                            e = pe.tile([P, PAN], bf16, tag="e")
                            nc.scalar.activation(e, ps_s, AF.Exp)
                            if kb >= 4 * g:
                                nc.vector.tensor_mul(
                                    e, e, masks_sb[:, kb - 4 * g, :]
                                )
                            nc.tensor.matmul(
                                ps_o, lhsT=v_sb[:, kb, h * P : (h + 1) * P], rhs=e,
                                start=(kb == 0), stop=(kb == nk - 1),
                            )
                            nc.tensor.matmul(
                                ps_d, lhsT=onc_sb[:], rhs=e,
                                start=(kb == 0), stop=(kb == nk - 1),
                            )
                        rc = work.tile([1, PAN], f32, tag="rc")
                        nc.vector.reciprocal(rc, ps_d)
                        rcb = work.tile([1, PAN], bf16, tag="rcb")
                        nc.vector.tensor_copy(rcb, rc)
                        ps_b = psBr.tile([P, PAN], f32, tag="ps_b")
                        nc.tensor.matmul(
                            ps_b, lhsT=onr_sb[:], rhs=rcb, start=True, stop=True
                        )
                        bb = work.tile([P, PAN], f32, tag="bb")
                        nc.scalar.copy(bb, ps_b)
                        ao_st = work.tile([P, PAN], bf16, tag="ao_st")
                        nc.vector.tensor_mul(ao_st, ps_o, bb)
                        nc.sync.dma_start(ao_dram[h * P : (h + 1) * P, gs], ao_st)

            # ---------------- Phase D: output projection -----------------
            with (
                tc.tile_pool(name="pd", bufs=1) as pd,
                tc.tile_pool(name="pda", bufs=2) as pda,
                tc.tile_pool(name="psD", bufs=4, space="PSUM") as psD,
            ):
                wo_sb = pd.tile([P, HL, D], bf16, tag="wo")
                nc.sync.dma_start(wo_sb[:], wo.rearrange("(kt p) m -> p kt m", p=P))
                aog = ao_dram.rearrange("(h p) s -> p h s", p=P)
                for m in range(S // P):
                    ms = slice(m * P, (m + 1) * P)
                    ao_sb = pda.tile([P, HL, P], bf16, tag="ao_rd")
                    nc.sync.dma_start(ao_sb[:], aog[:, :, ms])
                    for nn in range(D // PAN):
                        ps = psD.tile([P, PAN], f32, tag="psD")
                        for kt in range(HL):
                            nc.tensor.matmul(
                                ps,
                                lhsT=ao_sb[:, kt, :],
                                rhs=wo_sb[:, kt, nn * PAN : (nn + 1) * PAN],
                                start=(kt == 0), stop=(kt == HL - 1),
                            )
                        yst = work.tile([P, PAN], f32, tag="y_st")
                        nc.vector.tensor_copy(yst, ps)
                        nc.sync.dma_start(y[ms, nn * PAN : (nn + 1) * PAN], yst)

    nc.compile()
    return nc


def _prep_inputs(x, positions, Wq_down, Wq_up, Wq_rope, Wkv_down, Wk_up, Wv_up,
                 Wk_rope, Wo):
    scale = np.float32(QK_D ** -0.5)
    bf = lambda a: np.ascontiguousarray(a).astype(BF16)

    shared = {
        "wkr": bf(Wk_rope.T),
        "onc": np.ones((P, 1), BF16),
        "onr": np.ones((1, P), BF16),
    }
    inv_freq = 1.0 / (10000.0 ** (np.arange(0, RD, 2, dtype=np.float32) / RD))
    ang = positions.astype(np.float32)[:, None] * inv_freq  # (S, 32)
    cosT = np.ascontiguousarray(np.cos(ang).T).astype(np.float32)
    sinT = np.ascontiguousarray(np.sin(ang).T).astype(np.float32)
    shared["cosT"] = cosT
    shared["sinT"] = sinT

    mk = np.zeros((P, G * PAN), np.float32)
    for j in range(G):
        p = np.arange(P)[:, None]
        q = np.arange(PAN)[None, :]
        mk[:, j * PAN : (j + 1) * PAN] = (j * P + p <= q).astype(np.float32)
    shared["masks"] = mk.astype(BF16)

    wqdT = Wq_down.T  # (D, QR)
    wkvdT = Wkv_down.T  # (D, KVR)
    per_g = []
    for g in range(G):
        rs, rr = slice(512 * g, 512 * (g + 1)), slice(256 * g, 256 * (g + 1))
        per_g.append({
            "wqd": bf(wqdT[:, QSH * g : QSH * (g + 1)]),
            "wkvd": bf(wkvdT[:, KSH * g : KSH * (g + 1)]),
            "wqall": bf(np.concatenate(
                [(Wq_up[rs] * scale).T, (Wq_rope[rr] * scale).T], axis=1)),
            "wku": bf(Wk_up[rs].T),
            "wvu": bf(Wv_up[rs].T),
            "wo": bf(Wo[:, rs].T),
            "coskr": np.ascontiguousarray(cosT[:, PAN * g : PAN * (g + 1)]),
            "sinkr": np.ascontiguousarray(sinT[:, PAN * g : PAN * (g + 1)]),
        })
    xT = [bf(x[b].T) for b in range(B)]

    in_maps = []
    for c in range(8):
        b, g = c // G, c % G
        m = dict(shared)
        m.update(per_g[g])
        m["xT"] = xT[b]
        m["xkr"] = np.ascontiguousarray(xT[b][:, PAN * g : PAN * (g + 1)])
        in_maps.append(m)
    return in_maps


def kernel(**inputs):
    from concourse.bass_utils import run_bass_kernel_spmd

    if "nc" not in _cache:
        _cache["nc"] = _build_module()
    nc = _cache["nc"]

    in_maps = _prep_inputs(**inputs)
    res = run_bass_kernel_spmd(nc, in_maps, core_ids=list(range(8)))
    out = np.zeros((B, S, D), np.float32)
    for c in range(8):
        out[c // G] += res.results[c]["y"]
    return out


# revision 16
# speedup vs baseline: 1.3684x; 1.1367x over previous
"""Trainium2 Bass kernel for MultiLatentAttention (MLA).

Sharding: 8 cores = 2 (batch) x 4 (head-groups of 4 heads).
Within each batch group of 4 cores, the down-projections are sharded by
output rows and AllGathered (per S-panel, pipelined); the shared k_rope
head is sharded by S-panel and gathered once early.  Each core then runs
its 4 heads' up-projections + SDPA and a partial output projection
y_part = attn_out @ Wo[:, heads].T.  Host sums the 4 partials per batch.

On-device layout is feature-major ("transposed"): activations are [feat, S]
so every matmul contracts along the partition dim with zero transposes.
Scores are computed transposed [k, q]; softmax denominator comes from a
ones-vector matmul; normalization uses a K=1 broadcast matmul.
All matmul operands are bf16 (f32 PSUM accumulation).
"""

import sys

if "/opt/trn_rl_repo" not in sys.path:
    sys.path.insert(0, "/opt/trn_rl_repo")

import numpy as np
import ml_dtypes

BF16 = ml_dtypes.bfloat16

B, S, D, H = 2, 2048, 2048, 16
QR, KVR = 1536, 512
NOPE, RD, VD = 128, 64, 128
QK_D = NOPE + RD
HL = 4          # heads per core
G = 4           # head groups (= cores per batch group)
QSH = QR // G   # 384 c_q rows per core
KSH = KVR // G  # 128 c_kv rows per core
PAN = 512       # panel width
P = 128

_cache = {}


def _build_module(reps=1, phases="ABCD"):
    import concourse.bacc as bacc
    import concourse.mybir as mybir
    import concourse.tile as tile

    dt = mybir.dt
    f32, bf16 = dt.float32, dt.bfloat16
    AF = mybir.ActivationFunctionType

    nc = bacc.Bacc("TRN2", target_bir_lowering=False, debug=False, num_devices=8)

    def inp(name, shape, dtype=bf16):
        return nc.dram_tensor(name, shape, dtype, kind="ExternalInput").ap()

    xT = inp("xT", [D, S])                  # x[b].T
    xkr = inp("xkr", [D, PAN])              # x[b].T[:, my panel]
    wqd = inp("wqd", [D, QSH])              # Wq_down.T column slice
    wkvd = inp("wkvd", [D, KSH])            # Wkv_down.T column slice
    wkr = inp("wkr", [D, RD])               # Wk_rope.T
    wqall = inp("wqall", [QR, 768])         # [Wq_up_g.T*s | Wq_rope_g.T*s]
    wku = inp("wku", [KVR, 512])            # Wk_up_g.T
    wvu = inp("wvu", [KVR, 512])            # Wv_up_g.T
    wo = inp("wo", [512, D])                # Wo[:, cols_g].T
    cosT = inp("cosT", [32, S], f32)
    sinT = inp("sinT", [32, S], f32)
    coskr = inp("coskr", [32, PAN], f32)    # cos/sin for my k_rope panel
    sinkr = inp("sinkr", [32, PAN], f32)
    masks = inp("masks", [P, 4 * PAN])      # multiplicative causal masks
    onc = inp("onc", [P, 1])                # ones column
    y = nc.dram_tensor("y", [S, D], f32, kind="ExternalOutput").ap()

    KT_D = D // P      # 16 k-tiles over model dim
    KT_QR = QR // P    # 12
    KT_KV = KVR // P   # 4
    NP = S // PAN      # 4 panels
    GROUPS = [[0, 1, 2, 3], [4, 5, 6, 7]]

    with tile.TileContext(nc) as tc:
      for _rep in range(reps):
        with (
            tc.tile_pool(name="res", bufs=1) as res,
            tc.tile_pool(name="panels", bufs=2) as panels,
            tc.tile_pool(name="work", bufs=2) as work,
            tc.tile_pool(name="dram", bufs=1, space="DRAM") as dram,
        ):
            # ---- SBUF residents for SDPA --------------------------------
            qn_sb = res.tile([P, HL, S], bf16, tag="qn")
            qr_sb = res.tile([64, HL, S], bf16, tag="qr")
            k_c_sb = res.tile([P, HL, S], bf16, tag="k_c")
            v_sb = res.tile([P, S // P, 512], bf16, tag="v")
            k_r_sb = res.tile([64, NP, PAN], bf16, tag="k_r")
            masks_sb = res.tile([P, G, PAN], bf16, tag="masks")
            onc_sb = res.tile([P, 1], bf16, tag="onc")
            nc.sync.dma_start(masks_sb[:], masks.rearrange("p (j q) -> p j q", q=PAN))
            nc.sync.dma_start(onc_sb[:], onc[:])

            # ---- DRAM staging -------------------------------------------
            ag_in = [dram.tile([QSH + KSH, PAN], bf16, tag=f"agi{n}", name=f"agi{n}")
                     for n in range(NP)]
            ag_out = [dram.tile([G * (QSH + KSH), PAN], bf16, tag=f"ago{n}",
                                name=f"ago{n}") for n in range(NP)]
            kr_in = dram.tile([64, PAN], bf16, tag="kri", name="kri")
            kr_out = dram.tile([G * 64, PAN], bf16, tag="kro", name="kro")
            ao_dram = [dram.tile([HL * P, PAN], bf16, tag=f"aod{g}",
                                 name=f"aod{g}") for g in range(NP)]

            def rope_block(dst64, src64, cs, sn):
                # dst/src are [64, PAN]; rows 0:32 = first half dims
                t1 = work.tile([32, PAN], f32, tag="rope_t1")
                t2 = work.tile([32, PAN], f32, tag="rope_t2")
                nc.vector.tensor_mul(t1, src64[0:32, :], cs)
                nc.vector.tensor_mul(t2, src64[32:64, :], sn)
                nc.vector.tensor_sub(dst64[0:32, :], t1, t2)
                t3 = work.tile([32, PAN], f32, tag="rope_t1")
                t4 = work.tile([32, PAN], f32, tag="rope_t2")
                nc.vector.tensor_mul(t3, src64[32:64, :], cs)
                nc.vector.tensor_mul(t4, src64[0:32, :], sn)
                nc.vector.tensor_add(dst64[32:64, :], t3, t4)

            # ---- Phase A + B, panel-interleaved -------------------------
            with (
                tc.tile_pool(name="pa", bufs=1) as pa,
                tc.tile_pool(name="pb", bufs=1) as pb,
                tc.tile_pool(name="pbc", bufs=2) as pbc,
                tc.tile_pool(name="psA", bufs=3, space="PSUM") as psA,
                tc.tile_pool(name="psB", bufs=3, space="PSUM") as psB,
            ):
                # -- k_rope for my panel first, so the small gather clears early
                xkr_sb = panels.tile([P, KT_D, PAN], bf16, tag="panel", name="xkr_sb")
                nc.sync.dma_start(xkr_sb[:], xkr.rearrange("(kt p) s -> p kt s", p=P))
                wkr_sb = pa.tile([P, KT_D, RD], bf16, tag="wkr")
                nc.sync.dma_start(wkr_sb[:], wkr.rearrange("(kt p) m -> p kt m", p=P))
                ckr_sb = pa.tile([32, PAN], f32, tag="ckr")
                skr_sb = pa.tile([32, PAN], f32, tag="skr")
                nc.sync.dma_start(ckr_sb[:], coskr[:])
                nc.sync.dma_start(skr_sb[:], sinkr[:])
                ps = psA.tile([64, PAN], f32, tag="psKR", bufs=1)
                for kt in range(KT_D):
                    nc.tensor.matmul(
                        ps, lhsT=wkr_sb[:, kt, :], rhs=xkr_sb[:, kt, :],
                        start=(kt == 0), stop=(kt == KT_D - 1),
                    )
                krst = work.tile([64, PAN], bf16, tag="krst", bufs=1)
                rope_block(krst, ps, ckr_sb, skr_sb)
                nc.sync.dma_start(kr_in[:], krst)
                nc.gpsimd.collective_compute(
                    "AllGather", mybir.AluOpType.bypass,
                    replica_groups=GROUPS,
                    ins=[kr_in.opt()], outs=[kr_out.opt()],
                )
                nc.sync.dma_start(
                    k_r_sb[:], kr_out.rearrange("(g d) s -> d g s", d=64)
                )

                # -- A weights
                wqd_sb = pa.tile([P, KT_D, QSH], bf16, tag="wqd")
                nc.sync.dma_start(wqd_sb[:], wqd.rearrange("(kt p) m -> p kt m", p=P))
                wkvd_sb = pa.tile([P, KT_D, KSH], bf16, tag="wkvd")
                nc.sync.dma_start(wkvd_sb[:], wkvd.rearrange("(kt p) m -> p kt m", p=P))

                def phase_a(n):
                    """my slices of c_q / c_kv for panel n, then gather"""
                    ns = slice(n * PAN, (n + 1) * PAN)
                    x_sb = panels.tile([P, KT_D, PAN], bf16, tag="panel",
                                       name=f"x_sb{n}")
                    nc.sync.dma_start(
                        x_sb[:], xT[:, ns].rearrange("(kt p) s -> p kt s", p=P)
                    )
                    for m in range(QSH // P):  # 3 c_q row-tiles
                        ps = psA.tile([P, PAN], f32, tag="psA")
                        for kt in range(KT_D):
                            nc.tensor.matmul(
                                ps,
                                lhsT=wqd_sb[:, kt, m * P : (m + 1) * P],
                                rhs=x_sb[:, kt, :],
                                start=(kt == 0), stop=(kt == KT_D - 1),
                            )
                        st = work.tile([P, PAN], bf16, tag="cq_st")
                        nc.vector.tensor_copy(st, ps)
                        nc.sync.dma_start(ag_in[n][m * P : (m + 1) * P, :], st)
                    ps = psA.tile([P, PAN], f32, tag="psA")  # 1 c_kv row-tile
                    for kt in range(KT_D):
                        nc.tensor.matmul(
                            ps, lhsT=wkvd_sb[:, kt, :], rhs=x_sb[:, kt, :],
                            start=(kt == 0), stop=(kt == KT_D - 1),
                        )
                    st = work.tile([P, PAN], bf16, tag="cq_st")
                    nc.vector.tensor_copy(st, ps)
                    nc.sync.dma_start(ag_in[n][QSH : QSH + KSH, :], st)
                    nc.gpsimd.collective_compute(
                        "AllGather", mybir.AluOpType.bypass,
                        replica_groups=GROUPS,
                        ins=[ag_in[n].opt()], outs=[ag_out[n].opt()],
                    )

                def phase_b(n):
                    """up-projections for panel n from the gathered latents"""
                    ns = slice(n * PAN, (n + 1) * PAN)
                    # gathered latents: [(gi r p), s] with r=0..2 c_q, r=3 c_kv
                    gat = ag_out[n].rearrange("(gi r p) s -> p gi r s", p=P, r=4)
                    cq_sb = panels.tile([P, G, 3, PAN], bf16, tag="panel",
                                        name=f"cq_sb{n}")
                    for gi in range(G):
                        nc.sync.dma_start(cq_sb[:, gi, :, :], gat[:, gi, 0:3, :])
                    ckv_sb = pbc.tile([P, KT_KV, PAN], bf16, tag="ckv")
                    nc.sync.dma_start(ckv_sb[:], gat[:, :, 3, :])
                    cosp = pbc.tile([32, PAN], f32, tag="cosp", bufs=1)
                    sinp = pbc.tile([32, PAN], f32, tag="sinp", bufs=1)
                    nc.sync.dma_start(cosp[:], cosT[:, ns])
                    nc.sync.dma_start(sinp[:], sinT[:, ns])
                    for m in range(4):  # q nope heads
                        ps = psB.tile([P, PAN], f32, tag="psB")
                        for kt in range(KT_QR):
                            nc.tensor.matmul(
                                ps,
                                lhsT=wqall_sb[:, kt, m * P : (m + 1) * P],
                                rhs=cq_sb[:, kt // 3, kt % 3, :],
                                start=(kt == 0), stop=(kt == KT_QR - 1),
                            )
                        nc.vector.tensor_copy(qn_sb[:, m, ns], ps)
                    # rope heads: one M=64 matmul per head so each lands
                    # at base partition 0 (PE requires matching bases)
                    for h in range(HL):
                        c0 = 512 + 64 * h
                        ps = psB.tile([64, PAN], f32, tag="psBr64", bufs=1)
                        for kt in range(KT_QR):
                            nc.tensor.matmul(
                                ps,
                                lhsT=wqall_sb[:, kt, c0 : c0 + 64],
                                rhs=cq_sb[:, kt // 3, kt % 3, :],
                                start=(kt == 0), stop=(kt == KT_QR - 1),
                            )
                        rope_block(qr_sb[:, h, ns], ps, cosp, sinp)
                    # k_c for this panel
                    for m in range(HL):
                        ps = psB.tile([P, PAN], f32, tag="psB")
                        for kt in range(KT_KV):
                            nc.tensor.matmul(
                                ps,
                                lhsT=wku_sb[:, kt, m * P : (m + 1) * P],
                                rhs=ckv_sb[:, kt, :],
                                start=(kt == 0), stop=(kt == KT_KV - 1),
                            )
                        nc.vector.tensor_copy(k_c_sb[:, m, ns], ps)
                    # v for this panel's S-tiles
                    for sti in range(4):
                        st = 4 * n + sti
                        ps = psB.tile([P, PAN], f32, tag="psB")
                        for kt in range(KT_KV):
                            nc.tensor.matmul(
                                ps,
                                lhsT=ckv_sb[:, kt, sti * P : (sti + 1) * P],
                                rhs=wvu_sb[:, kt, :],
                                start=(kt == 0), stop=(kt == KT_KV - 1),
                            )
                        nc.vector.tensor_copy(v_sb[:, st, :], ps)

                # interleave emission so the shared panel slots rotate A/B/A/B
                phase_a(0)
                # -- B weights (gpsimd DMA queue, off the hot SP queue)
                wqall_sb = pb.tile([P, KT_QR, 768], bf16, tag="wqall")
                nc.gpsimd.dma_start(wqall_sb[:], wqall.rearrange("(kt p) m -> p kt m", p=P))
                wku_sb = pb.tile([P, KT_KV, 512], bf16, tag="wku")
                nc.gpsimd.dma_start(wku_sb[:], wku.rearrange("(kt p) m -> p kt m", p=P))
                wvu_sb = pb.tile([P, KT_KV, 512], bf16, tag="wvu")
                nc.gpsimd.dma_start(wvu_sb[:], wvu.rearrange("(kt p) m -> p kt m", p=P))
                phase_a(1)
                phase_b(0)
                phase_a(2)
                phase_b(1)
                phase_a(3)
                phase_b(2)
                phase_b(3)

            # ---------------- Phase C: SDPA + Phase D interleaved --------
            if "C" not in phases:
                # timing-partial build: consume B outputs so nothing is elided
                nc.gpsimd.dma_start(y[0:P, 0:PAN], qn_sb[:, 0, 0:PAN])
                nc.gpsimd.dma_start(y[P : 2 * P, 0:PAN], k_c_sb[:, 0, 0:PAN])
                nc.gpsimd.dma_start(y[2 * P : 3 * P, 0:PAN], v_sb[:, 0, 0:PAN])
                nc.gpsimd.dma_start(y[3 * P : 3 * P + 64, 0:PAN], qr_sb[:, 0, 0:PAN])
                nc.gpsimd.dma_start(y[4 * P : 4 * P + 64, 0:PAN], k_r_sb[:, 0, :])
                continue
            with (
                tc.tile_pool(name="pe", bufs=4) as pe,
                tc.tile_pool(name="pd", bufs=1) as pd,
                tc.tile_pool(name="pda", bufs=2) as pda,
                tc.tile_pool(name="psS", bufs=3, space="PSUM") as psS,
                tc.tile_pool(name="psO", bufs=2, space="PSUM") as psO,
                tc.tile_pool(name="psDn", bufs=2, space="PSUM") as psDn,
                tc.tile_pool(name="psD", bufs=1, space="PSUM") as psD,
            ):
                wo_sb = pd.tile([P, HL, D], bf16, tag="wo")
                nc.gpsimd.dma_start(wo_sb[:], wo.rearrange("(kt p) m -> p kt m", p=P))

                def phase_d(m):
                    if "D" not in phases:
                        return
                    ms = slice(m * P, (m + 1) * P)
                    g = m // 4
                    aog = ao_dram[g].rearrange("(h p) s -> p h s", p=P)
                    ao_sb = pda.tile([P, HL, P], bf16, tag="ao_rd")
                    nc.sync.dma_start(
                        ao_sb[:], aog[:, :, (m % 4) * P : (m % 4 + 1) * P]
                    )
                    for nn in range(D // PAN):
                        ps = psD.tile([P, PAN], f32, tag="psD")
                        for kt in range(HL):
                            nc.tensor.matmul(
                                ps,
                                lhsT=ao_sb[:, kt, :],
                                rhs=wo_sb[:, kt, nn * PAN : (nn + 1) * PAN],
                                start=(kt == 0), stop=(kt == HL - 1),
                            )
                        yst = work.tile([P, PAN], f32, tag="y_st")
                        nc.vector.tensor_copy(yst, ps)
                        nc.sync.dma_start(y[ms, nn * PAN : (nn + 1) * PAN], yst)

                for g in range(G):
                    gs = slice(g * PAN, (g + 1) * PAN)
                    for h in range(HL):
                        qn = qn_sb[:, h, gs]
                        qr = qr_sb[:, h, gs]
                        ps_o = psO.tile([P, PAN], f32, tag="ps_o")
                        ps_d = psDn.tile([1, PAN], f32, tag="ps_d")
                        nk = 4 * (g + 1)
                        for kb in range(nk):
                            ks = slice(kb * P, (kb + 1) * P)
                            ps_s = psS.tile([P, PAN], f32, tag="ps_s")
                            nc.tensor.matmul(
                                ps_s, lhsT=k_c_sb[:, h, ks], rhs=qn,
                                start=True, stop=False,
                            )
                            nc.tensor.matmul(
                                ps_s,
                                lhsT=k_r_sb[:, kb // 4, (kb % 4) * P : (kb % 4 + 1) * P],
                                rhs=qr,
                                start=False, stop=True,
                            )
                            e = pe.tile([P, PAN], bf16, tag="e")
                            nc.scalar.activation(e, ps_s, AF.Exp)
                            if kb >= 4 * g:
                                nc.vector.tensor_mul(
                                    e, e, masks_sb[:, kb - 4 * g, :]
                                )
                            nc.tensor.matmul(
                                ps_o, lhsT=v_sb[:, kb, h * P : (h + 1) * P], rhs=e,
                                start=(kb == 0), stop=(kb == nk - 1),
                            )
                            nc.tensor.matmul(
                                ps_d, lhsT=onc_sb[:], rhs=e,
                                start=(kb == 0), stop=(kb == nk - 1),
                            )
                        rc = work.tile([1, PAN], f32, tag="rc")
                        nc.vector.reciprocal(rc, ps_d)
                        bb = work.tile([P, PAN], f32, tag="bb")
                        nc.gpsimd.partition_broadcast(bb, rc)
                        ao_st = work.tile([P, PAN], bf16, tag="ao_st")
                        nc.vector.tensor_mul(ao_st, ps_o, bb)
                        nc.sync.dma_start(ao_dram[g][h * P : (h + 1) * P, :], ao_st)
                    for m in range(4 * g, 4 * g + 4):
                        phase_d(m)

    nc.compile()
    return nc


def _prep_inputs(x, positions, Wq_down, Wq_up, Wq_rope, Wkv_down, Wk_up, Wv_up,
                 Wk_rope, Wo):
    scale = np.float32(QK_D ** -0.5)
    bf = lambda a: np.ascontiguousarray(a).astype(BF16)

    shared = {
        "wkr": bf(Wk_rope.T),
        "onc": np.ones((P, 1), BF16),
    }
    inv_freq = 1.0 / (10000.0 ** (np.arange(0, RD, 2, dtype=np.float32) / RD))
    ang = positions.astype(np.float32)[:, None] * inv_freq  # (S, 32)
    cosT = np.ascontiguousarray(np.cos(ang).T).astype(np.float32)
    sinT = np.ascontiguousarray(np.sin(ang).T).astype(np.float32)
    shared["cosT"] = cosT
    shared["sinT"] = sinT

    mk = np.zeros((P, G * PAN), np.float32)
    for j in range(G):
        p = np.arange(P)[:, None]
        q = np.arange(PAN)[None, :]
        mk[:, j * PAN : (j + 1) * PAN] = (j * P + p <= q).astype(np.float32)
    shared["masks"] = mk.astype(BF16)

    wqdT = Wq_down.T  # (D, QR)
    wkvdT = Wkv_down.T  # (D, KVR)
    per_g = []
    for g in range(G):
        rs, rr = slice(512 * g, 512 * (g + 1)), slice(256 * g, 256 * (g + 1))
        per_g.append({
            "wqd": bf(wqdT[:, QSH * g : QSH * (g + 1)]),
            "wkvd": bf(wkvdT[:, KSH * g : KSH * (g + 1)]),
            "wqall": bf(np.concatenate(
                [(Wq_up[rs] * scale).T, (Wq_rope[rr] * scale).T], axis=1)),
            "wku": bf(Wk_up[rs].T),
            "wvu": bf(Wv_up[rs].T),
            "wo": bf(Wo[:, rs].T),
            "coskr": np.ascontiguousarray(cosT[:, PAN * g : PAN * (g + 1)]),
            "sinkr": np.ascontiguousarray(sinT[:, PAN * g : PAN * (g + 1)]),
        })
    xT = [bf(x[b].T) for b in range(B)]

    in_maps = []
    for c in range(8):
        b, g = c // G, c % G
        m = dict(shared)
        m.update(per_g[g])
        m["xT"] = xT[b]
        m["xkr"] = np.ascontiguousarray(xT[b][:, PAN * g : PAN * (g + 1)])
        in_maps.append(m)
    return in_maps


def kernel(**inputs):
    from concourse.bass_utils import run_bass_kernel_spmd

    if "nc" not in _cache:
        _cache["nc"] = _build_module()
    nc = _cache["nc"]

    in_maps = _prep_inputs(**inputs)
    res = run_bass_kernel_spmd(nc, in_maps, core_ids=list(range(8)))
    out = np.zeros((B, S, D), np.float32)
    for c in range(8):
        out[c // G] += res.results[c]["y"]
    return out
